# revision 4
# baseline (speedup 1.0000x reference)
import os

_flags = os.environ.get('NEURON_CC_FLAGS', '')
if '--auto-cast' not in _flags:
    os.environ['NEURON_CC_FLAGS'] = (_flags + ' --auto-cast=none').strip()

import numpy as np
import jax
import jax.numpy as jnp
from jax import lax

# ---- hardcoded problem config (nn_Pointnet2_16776142258207) ----
B, N, NUM_CLASSES = 16, 4096, 13
N_CORES = 8
SHARD = B // N_CORES  # 2 clouds per core
SA_CFG = [(1024, 0.1, 32, [9, 32, 32, 64]),
          (256, 0.2, 32, [67, 64, 64, 128]),
          (64, 0.4, 32, [131, 128, 128, 256]),
          (16, 0.8, 32, [259, 256, 256, 512])]
FP_DIMS = [[768, 256, 256], [384, 256, 256], [320, 256, 128], [134, 128, 128, 128]]

# FPS chunk schedule per level: (npoint, chunk) — chunk sizes keep each
# compiled module small (no `while` support in neuronx-cc; fully unrolled).
FPS_CHUNK = {1024: 128, 256: 128, 64: 64, 16: 16}


def _gather(p, i):
    return jax.vmap(lambda a, b: a[b])(p, i)


def _sqdist(a, b):
    return (jnp.sum(a * a, -1)[:, :, None] + jnp.sum(b * b, -1)[:, None, :]
            - 2.0 * jnp.einsum('bnc,bmc->bnm', a, b))


# ---------- FPS: chunked unrolled steps, state stays on device ----------
_fps_cache = {}


def _fps_chunk_fn(nchunk, n_pts):
    key = (nchunk, n_pts)
    if key in _fps_cache:
        return _fps_cache[key]

    def f(xyz, dists, last):
        # xyz [S, n, 3], dists [S, n], last [S] int32
        iota = jnp.arange(n_pts, dtype=jnp.int32)[None, :]
        outs = []
        for _ in range(nchunk):
            lx = jnp.take_along_axis(xyz, last[:, None, None], axis=1)
            d = jnp.sum((xyz - lx) ** 2, -1)
            dists = jnp.minimum(dists, d)
            m = jnp.max(dists, -1, keepdims=True)
            outs.append(last)
            last = jnp.min(jnp.where(dists >= m, iota, n_pts), -1).astype(jnp.int32)
        return dists, last, jnp.stack(outs, 1)

    pf = jax.pmap(f)
    _fps_cache[key] = pf
    return pf


def _fps_device(xyz_np):
    # xyz_np [8, SHARD, n, 3] numpy -> idx [8, SHARD, npoint] numpy
    n = xyz_np.shape[2]
    npoint = {4096: 1024, 1024: 256, 256: 64, 64: 16}[n]
    chunk = FPS_CHUNK[npoint]
    pf = _fps_chunk_fn(chunk, n)
    xyz_d = jnp.asarray(xyz_np)
    dists = jnp.full((N_CORES, SHARD, n), 1e10, jnp.float32)
    last = jnp.zeros((N_CORES, SHARD), jnp.int32)
    idxs = []
    for _ in range(npoint // chunk):
        dists, last, idx = pf(xyz_d, dists, last)
        idxs.append(idx)
    return np.concatenate([np.asarray(i) for i in idxs], axis=2)


# ---------- main network (new_xyz precomputed, single pmap) ----------
def _ball_query(radius, nsample, xyz, new_xyz):
    d2 = _sqdist(new_xyz, xyz)
    Nc = xyz.shape[1]
    cand = jnp.where(d2 < radius * radius, jnp.arange(Nc, dtype=jnp.float32),
                     jnp.float32(Nc))
    idx = (-lax.top_k(-cand, nsample)[0]).astype(jnp.int32)
    return jnp.where(idx == Nc, idx[..., :1], idx)


def _mlp(x, layers):
    for l in layers:
        x = jax.nn.relu((x @ l['W'] + l['b']) * l['g'] + l['be'])
    return x


def _sa(xyz, feats, new_xyz, radius, nsample, layers):
    idx = _ball_query(radius, nsample, xyz, new_xyz)
    g = _gather(xyz, idx) - new_xyz[:, :, None, :]
    if feats is not None:
        g = jnp.concatenate([g, _gather(feats, idx)], axis=-1)
    return jnp.max(_mlp(g, layers), axis=2)


def _fp(xyz1, xyz2, feats1, feats2, layers):
    negd, idx = lax.top_k(-_sqdist(xyz1, xyz2), 3)
    w = 1.0 / (-negd + 1e-8)
    w = w / jnp.sum(w, -1, keepdims=True)
    interp = jnp.sum(_gather(feats2, idx) * w[..., None], axis=2)
    return _mlp(jnp.concatenate([interp, feats1], axis=-1), layers)


def _forward_main(xyz, points, nx1, nx2, nx3, nx4, params):
    xs = [xyz, nx1, nx2, nx3, nx4]
    fs = [points]
    for i, (npoint, radius, nsample, _) in enumerate(SA_CFG):
        nf = _sa(xs[i], fs[i], xs[i + 1], radius, nsample, params['sa%d' % i])
        fs.append(nf)
    f = fs[4]
    for i in range(4):
        lvl = 3 - i
        f = _fp(xs[lvl], xs[lvl + 1], fs[lvl], f, params['fp%d' % i])
    h = _mlp(f, params['fc'])
    return h @ params['out']['W'] + params['out']['b']


_pmain = jax.pmap(_forward_main, in_axes=(0, 0, 0, 0, 0, 0, None))


def kernel(xyz, points, params):
    xyz = np.asarray(xyz, dtype=np.float32)
    points = np.asarray(points, dtype=np.float32)
    params = jax.tree_util.tree_map(lambda a: jnp.asarray(a, jnp.float32), params)
    xs0 = xyz.reshape(N_CORES, SHARD, N, 3)
    ps = points.reshape(N_CORES, SHARD, N, 6)

    # FPS cascade: device computes indices, host composes new_xyz by gathering
    nxs = []
    cur = xs0
    for lvl in range(4):
        idx = _fps_device(cur)  # [8, SHARD, npoint]
        cur = np.take_along_axis(cur, idx[..., None], axis=2)
        nxs.append(cur)

    out = _pmain(jnp.asarray(xs0), jnp.asarray(ps),
                 jnp.asarray(nxs[0]), jnp.asarray(nxs[1]),
                 jnp.asarray(nxs[2]), jnp.asarray(nxs[3]), params)
    return np.asarray(out, dtype=np.float32).reshape(B, N, NUM_CLASSES)
